# revision 4
# baseline (speedup 1.0000x reference)
"""Trainium2 Bass kernel for nn_CS_MAMBA (pool -> mamba -> channel-attention -> FFN).

Data-parallel over batch: 64 batch items sharded 8-per-core across 8 NeuronCores;
all weights replicated. Everything fp32 except the two big FFN matmuls (bf16
operands, fp32 PSUM accumulation).
"""

import numpy as np
import ml_dtypes

# ---------------------------------------------------------------- constants
B_FULL = 64
N_CORES = 8
BL = B_FULL // N_CORES          # 8 batch items per core
C = 2048
NCT = C // 128                  # 16 channel tiles
H, W = 24, 12
HW = H * W                      # 288
POOL_W = 48                     # elements averaged per patch (4 rows x 12 cols)
L = 12                          # interleaved sequence length
COLS = L * BL                   # 96, column index = l*8 + b
DI = 256                        # d_inner
DIT = DI // 128                 # 2 d_inner tiles
DS = 16                         # d_state
DTR = 16                        # dt_rank
EPS = 1e-5

_CACHE = {}


def _build(nc_mod, tile_mod, mybir, masks):
    """Emit the bass program. Returns the compiled Bass object."""
    F32 = mybir.dt.float32
    BF16 = mybir.dt.bfloat16
    AF = mybir.ActivationFunctionType
    ALU = mybir.AluOpType
    AX = mybir.AxisListType

    nc = nc_mod.Bacc("TRN2", target_bir_lowering=False, debug=False)

    # ---------------- dram tensors (names = in_map keys)
    d_vis = nc.dram_tensor("vis", [BL, C, HW], F32, kind="ExternalInput")
    d_inf = nc.dram_tensor("inf", [BL, C, HW], F32, kind="ExternalInput")
    d_winT = nc.dram_tensor("w_inT", [128, NCT, DI], F32, kind="ExternalInput")
    d_wxT = nc.dram_tensor("wxT", [128, DIT, 48], F32, kind="ExternalInput")
    d_wdtT = nc.dram_tensor("wdtT", [DTR, DI], F32, kind="ExternalInput")
    d_woutT = nc.dram_tensor("w_outT", [128, DIT, C], F32, kind="ExternalInput")
    d_aw1T = nc.dram_tensor("aw1T", [128, NCT, 128], F32, kind="ExternalInput")
    d_aw2T = nc.dram_tensor("aw2T", [128, C], F32, kind="ExternalInput")
    d_A3 = nc.dram_tensor("A3", [128, DIT, DS], F32, kind="ExternalInput")
    d_cw3 = nc.dram_tensor("cw3", [128, DIT, 3], F32, kind="ExternalInput")
    d_ncb = nc.dram_tensor("ncb", [128, DIT], F32, kind="ExternalInput")
    d_bdt = nc.dram_tensor("bdt2", [128, DIT], F32, kind="ExternalInput")
    d_dssm = nc.dram_tensor("dssm2", [128, DIT], F32, kind="ExternalInput")
    d_ln1g = nc.dram_tensor("ln1g", [128, NCT], F32, kind="ExternalInput")
    d_ln1b = nc.dram_tensor("ln1b", [128, NCT], F32, kind="ExternalInput")
    d_ln2g = nc.dram_tensor("ln2g", [128, NCT], F32, kind="ExternalInput")
    d_ln2b = nc.dram_tensor("ln2b", [128, NCT], F32, kind="ExternalInput")
    d_absm = nc.dram_tensor("absm", [128, 1], F32, kind="ExternalInput")
    d_absx = nc.dram_tensor("absx", [128, 1], F32, kind="ExternalInput")
    d_abnb = nc.dram_tensor("abnb", [128, 1], F32, kind="ExternalInput")
    d_wvT = nc.dram_tensor("wvT", [128, NCT, C], BF16, kind="ExternalInput")
    d_wiT = nc.dram_tensor("wiT", [128, NCT, C], BF16, kind="ExternalInput")
    d_fvs = nc.dram_tensor("fvs", [128, NCT], F32, kind="ExternalInput")
    d_fvb = nc.dram_tensor("fvb", [128, NCT], F32, kind="ExternalInput")
    d_fis = nc.dram_tensor("fis", [128, NCT], F32, kind="ExternalInput")
    d_fib = nc.dram_tensor("fib", [128, NCT], F32, kind="ExternalInput")

    d_out_vis = nc.dram_tensor("out_vis", [BL, C, HW], F32, kind="ExternalOutput")
    d_out_inf = nc.dram_tensor("out_inf", [BL, C, HW], F32, kind="ExternalOutput")

    with tile_mod.TileContext(nc) as tc:
        with (
            tc.tile_pool(name="consts", bufs=1) as consts,
            tc.tile_pool(name="wpool", bufs=1) as wpool,
            tc.tile_pool(name="axpool", bufs=2) as axpool,
            tc.tile_pool(name="stream", bufs=8) as stream,
            tc.tile_pool(name="outp", bufs=4) as outp,
            tc.tile_pool(name="vip", bufs=1) as vip,
            tc.tile_pool(name="mam", bufs=1) as mam,
            tc.tile_pool(name="psA", bufs=3, space="PSUM") as psA,
            tc.tile_pool(name="psB", bufs=1, space="PSUM") as psB,
            tc.tile_pool(name="psC", bufs=2, space="PSUM") as psC,
        ):
            # ---------------- constants / weights to SBUF
            ident = consts.tile([128, 128], F32)
            masks.make_identity(nc, ident)
            ones_col = consts.tile([128, 1], F32)
            nc.vector.memset(ones_col, 1.0)
            ones_row = consts.tile([1, 128], F32)
            nc.vector.memset(ones_row, 1.0)
            epsv = consts.tile([128, 1], F32)
            nc.vector.memset(epsv, EPS)

            winT = consts.tile([128, NCT, DI], F32)
            nc.sync.dma_start(out=winT, in_=d_winT[:, :, :])
            wxT = consts.tile([128, DIT, 48], F32)
            nc.sync.dma_start(out=wxT, in_=d_wxT[:, :, :])
            wdtT = consts.tile([DTR, DI], F32)
            nc.sync.dma_start(out=wdtT, in_=d_wdtT[:, :])
            woutT = consts.tile([128, DIT, C], F32)
            nc.sync.dma_start(out=woutT, in_=d_woutT[:, :, :])
            aw1T = consts.tile([128, NCT, 128], F32)
            nc.sync.dma_start(out=aw1T, in_=d_aw1T[:, :, :])
            aw2T = consts.tile([128, C], F32)
            nc.sync.dma_start(out=aw2T, in_=d_aw2T[:, :])
            A3 = consts.tile([128, DIT, DS], F32)
            nc.sync.dma_start(out=A3, in_=d_A3[:, :, :])
            cw3 = consts.tile([128, DIT, 3], F32)
            nc.sync.dma_start(out=cw3, in_=d_cw3[:, :, :])
            ncb = consts.tile([128, DIT], F32)
            nc.sync.dma_start(out=ncb, in_=d_ncb[:, :])
            bdt2 = consts.tile([128, DIT], F32)
            nc.sync.dma_start(out=bdt2, in_=d_bdt[:, :])
            dssm2 = consts.tile([128, DIT], F32)
            nc.sync.dma_start(out=dssm2, in_=d_dssm[:, :])
            ln1g = consts.tile([128, NCT], F32)
            nc.sync.dma_start(out=ln1g, in_=d_ln1g[:, :])
            ln1b = consts.tile([128, NCT], F32)
            nc.sync.dma_start(out=ln1b, in_=d_ln1b[:, :])
            ln2g = consts.tile([128, NCT], F32)
            nc.sync.dma_start(out=ln2g, in_=d_ln2g[:, :])
            ln2b = consts.tile([128, NCT], F32)
            nc.sync.dma_start(out=ln2b, in_=d_ln2b[:, :])
            absm = consts.tile([128, 1], F32)
            nc.sync.dma_start(out=absm, in_=d_absm[:, :])
            absx = consts.tile([128, 1], F32)
            nc.sync.dma_start(out=absx, in_=d_absx[:, :])
            abnb = consts.tile([128, 1], F32)
            nc.sync.dma_start(out=abnb, in_=d_abnb[:, :])
            fvs = consts.tile([128, NCT], F32)
            nc.sync.dma_start(out=fvs, in_=d_fvs[:, :])
            fvb = consts.tile([128, NCT], F32)
            nc.sync.dma_start(out=fvb, in_=d_fvb[:, :])
            fis = consts.tile([128, NCT], F32)
            nc.sync.dma_start(out=fis, in_=d_fis[:, :])
            fib = consts.tile([128, NCT], F32)
            nc.sync.dma_start(out=fib, in_=d_fib[:, :])

            # attention weights (outputs of the mamba/attention front-end)
            att_vis = consts.tile([128, NCT, BL], F32)
            att_inf = consts.tile([128, NCT, BL], F32)

            fm_d = [d_vis, d_inf]
            out_d = [d_out_vis, d_out_inf]

            # ============================================================
            # Phase P: adaptive avg-pool (as SUM; /48 folded in later) into
            # Vi[128, ci, col] with col = l*8 + b, l = 2*p + s.
            # ============================================================
            Vi = vip.tile([128, NCT, COLS], F32)
            for b in range(BL):
                for s in range(2):
                    for ci in range(NCT):
                        ft = stream.tile(
                            [128, HW], F32, tag="fm", name=f"pfm{s}_{b}_{ci}"
                        )
                        nc.sync.dma_start(
                            out=ft, in_=fm_d[s][b, ci * 128 : (ci + 1) * 128, :]
                        )
                        nc.vector.reduce_sum(
                            out=Vi[:, ci, :].rearrange("p (pp x) -> p pp x", x=16)[
                                :, :, 8 * s + b
                            ],
                            in_=ft.rearrange("p (pp w) -> p pp w", w=POOL_W),
                            axis=AX.X,
                        )

            # ============================================================
            # Phase M: mamba block + channel attention (all 8 batch items).
            # Layout B: [c partitions, col = l*8+b].
            # ============================================================
            def layer_norm(src_tile, g_tile, b_tile, dst_tile):
                """LN over channels (partition dim across 16 tiles) for each col."""
                sq = mam.tile([128, NCT, COLS], F32, tag="lnsq", name="lnsq")
                for ci in range(NCT):
                    nc.scalar.activation(
                        out=sq[:, ci, :], in_=src_tile[:, ci, :], func=AF.Square
                    )
                s1p = psC.tile([128, COLS], F32, tag="ps96", name="s1p")
                s2p = psC.tile([128, COLS], F32, tag="ps96", name="s2p")
                for ci in range(NCT):
                    nc.tensor.matmul(
                        s1p[0:1, :], ones_col, src_tile[:, ci, :],
                        start=(ci == 0), stop=(ci == NCT - 1),
                    )
                    nc.tensor.matmul(
                        s2p[0:1, :], ones_col, sq[:, ci, :],
                        start=(ci == 0), stop=(ci == NCT - 1),
                    )
                m_sb = mam.tile([1, COLS], F32, tag="lnm", name="lnm")
                nc.vector.tensor_scalar_mul(m_sb, s1p[0:1, :], 1.0 / C)
                v_sb = mam.tile([1, COLS], F32, tag="lnv", name="lnv")
                # var = s2/C - m^2
                nc.vector.tensor_scalar_mul(v_sb, s2p[0:1, :], 1.0 / C)
                msq = mam.tile([1, COLS], F32, tag="lnmsq", name="lnmsq")
                nc.vector.tensor_mul(msq, m_sb, m_sb)
                nc.vector.tensor_sub(v_sb, v_sb, msq)
                # r = (var+eps)^-1/2 = exp(-0.5*ln(var+eps))
                r_sb = mam.tile([1, COLS], F32, tag="lnr", name="lnr")
                nc.scalar.activation(out=r_sb, in_=v_sb, func=AF.Ln, bias=epsv[0:1, :])
                nc.scalar.activation(out=r_sb, in_=r_sb, func=AF.Exp, scale=-0.5)
                mr_sb = mam.tile([1, COLS], F32, tag="lnmr", name="lnmr")
                nc.vector.tensor_mul(mr_sb, m_sb, r_sb)
                # broadcast r, m*r to 128 partitions
                rb = psC.tile([128, COLS], F32, tag="ps96", name="lnrb")
                nc.tensor.matmul(rb, ones_row, r_sb, start=True, stop=True)
                mrb = psC.tile([128, COLS], F32, tag="ps96", name="lnmrb")
                nc.tensor.matmul(mrb, ones_row, mr_sb, start=True, stop=True)
                for ci in range(NCT):
                    nc.vector.tensor_tensor(
                        out=dst_tile[:, ci, :], in0=src_tile[:, ci, :], in1=rb,
                        op=ALU.mult,
                    )
                    nc.vector.tensor_tensor(
                        out=dst_tile[:, ci, :], in0=dst_tile[:, ci, :], in1=mrb,
                        op=ALU.subtract,
                    )
                    nc.vector.tensor_scalar(
                        out=dst_tile[:, ci, :], in0=dst_tile[:, ci, :],
                        scalar1=g_tile[:, ci : ci + 1], scalar2=b_tile[:, ci : ci + 1],
                        op0=ALU.mult, op1=ALU.add,
                    )

            # ---- LN1 (scale-invariant: Vi holds 48x the true pooled values)
            xn = vip.tile([128, NCT, COLS], F32, tag="v96", bufs=2)
            layer_norm(Vi, ln1g, ln1b, xn)

            # ---- x = xn @ W_in.T  -> [256(d), 96] ;  conv along l + silu
            xact = mam.tile([128, DIT, COLS], F32)
            cv = mam.tile([128, DIT, COLS], F32)
            e_t = mam.tile([128, DIT, COLS], F32)
            for i in range(DIT):
                xp = psC.tile([128, COLS], F32, tag="ps96", name=f"xp{i}")
                for ci in range(NCT):
                    nc.tensor.matmul(
                        xp, winT[:, ci, i * 128 : (i + 1) * 128], xn[:, ci, :],
                        start=(ci == 0), stop=(ci == NCT - 1),
                    )
                # depthwise conv: cv = w1*x (+w0*x_{l-1} +w2*x_{l+1}); cols l*8+b
                nc.vector.tensor_scalar_mul(
                    out=cv[:, i, :], in0=xp, scalar1=cw3[:, i, 1:2]
                )
                x_sb = mam.tile([128, COLS], F32, tag="xsb", name=f"xsb{i}")
                nc.vector.tensor_copy(out=x_sb, in_=xp)
                nc.vector.scalar_tensor_tensor(
                    out=cv[:, i, 8:COLS], in0=x_sb[:, 0 : COLS - 8],
                    scalar=cw3[:, i, 0:1], in1=cv[:, i, 8:COLS],
                    op0=ALU.mult, op1=ALU.add,
                )
                nc.vector.scalar_tensor_tensor(
                    out=cv[:, i, 0 : COLS - 8], in0=x_sb[:, 8:COLS],
                    scalar=cw3[:, i, 2:3], in1=cv[:, i, 0 : COLS - 8],
                    op0=ALU.mult, op1=ALU.add,
                )
                # silu(cv + conv_b) = (cv+cb) / (1 + exp(-(cv+cb))); ncb = -conv_b
                nc.scalar.activation(
                    out=e_t[:, i, :], in_=cv[:, i, :], func=AF.Exp,
                    scale=-1.0, bias=ncb[:, i : i + 1],
                )
                nc.vector.tensor_scalar_add(
                    out=e_t[:, i, :], in0=e_t[:, i, :], scalar1=1.0
                )
                nc.vector.reciprocal(out=e_t[:, i, :], in_=e_t[:, i, :])
                nc.vector.scalar_tensor_tensor(
                    out=xact[:, i, :], in0=cv[:, i, :], scalar=ncb[:, i : i + 1],
                    in1=e_t[:, i, :], op0=ALU.subtract, op1=ALU.mult,
                )

            # ---- dbc = x @ Wx.T -> [96(l,b), 48]
            dbcp = psC.tile([128, 48], F32, tag="ps96", name="dbcp")
            for i in range(DIT):
                nc.tensor.matmul(
                    dbcp[0:COLS, :], xact[:, i, :], wxT[:, i, :],
                    start=(i == 0), stop=(i == DIT - 1),
                )
            dbc_sb = mam.tile([COLS, 48], F32)
            nc.vector.tensor_copy(out=dbc_sb, in_=dbcp[0:COLS, :])

            # ---- delta = softplus(delta_in @ Wdt.T + bdt) -> [256, 96]
            dtp = psC.tile([128, COLS], F32, tag="ps96", name="dtp")
            nc.tensor.transpose(dtp[0:DTR, :], dbc_sb[:, 0:DTR], ident[0:COLS, 0:COLS])
            dT_sb = mam.tile([DTR, COLS], F32)
            nc.vector.tensor_copy(out=dT_sb, in_=dtp[0:DTR, :])
            delta = mam.tile([128, DIT, COLS], F32)
            for i in range(DIT):
                dp = psC.tile([128, COLS], F32, tag="ps96", name=f"dp{i}")
                nc.tensor.matmul(
                    dp, wdtT[:, i * 128 : (i + 1) * 128], dT_sb,
                    start=True, stop=True,
                )
                nc.scalar.activation(
                    out=delta[:, i, :], in_=dp, func=AF.Exp,
                    bias=bdt2[:, i : i + 1],
                )
                nc.scalar.activation(
                    out=delta[:, i, :], in_=delta[:, i, :], func=AF.Ln, bias=1.0
                )

            # ---- dA = exp(delta x A): [128, i, (l,b,n)]
            dA = mam.tile([128, DIT, COLS * DS], BF16)
            for i in range(DIT):
                nc.vector.tensor_tensor(
                    out=dA[:, i, :].rearrange("p (l bb n) -> p l bb n", bb=BL, n=DS),
                    in0=delta[:, i, :]
                    .rearrange("p (l bb) -> p l bb", bb=BL)
                    .unsqueeze(3)
                    .broadcast_to([128, L, BL, DS]),
                    in1=A3[:, i, :]
                    .unsqueeze(1)
                    .unsqueeze(1)
                    .broadcast_to([128, L, BL, DS]),
                    op=ALU.mult,
                )
            nc.scalar.activation(out=dA[:, :, :], in_=dA[:, :, :], func=AF.Exp)

            # ---- dBu = (delta*x) x Bp : Bp gathered to [1,1536] then bcast
            du = mam.tile([128, DIT, COLS], F32)
            for i in range(DIT):
                nc.vector.tensor_mul(du[:, i, :], delta[:, i, :], xact[:, i, :])
            bp_flat = mam.tile([1, COLS * DS], F32, tag="flat", name="bp_flat")
            nc.sync.dma_start(
                out=bp_flat.rearrange("o (pb n) -> o pb n", n=DS),
                in_=dbc_sb[:, DTR : DTR + DS],
            )
            dBu = mam.tile([128, DIT, COLS * DS], BF16)
            bpb = psB.tile([128, COLS * DS], F32, tag="bc", name="bpb")
            for k in range(3):
                nc.tensor.matmul(
                    bpb[:, 512 * k : 512 * (k + 1)], ones_row,
                    bp_flat[:, 512 * k : 512 * (k + 1)], start=True, stop=True,
                )
            for i in range(DIT):
                nc.vector.tensor_tensor(
                    out=dBu[:, i, :].rearrange("p (l bb n) -> p l bb n", bb=BL, n=DS),
                    in0=du[:, i, :]
                    .rearrange("p (l bb) -> p l bb", bb=BL)
                    .unsqueeze(3)
                    .broadcast_to([128, L, BL, DS]),
                    in1=bpb.rearrange("p (l bb n) -> p l bb n", bb=BL, n=DS),
                    op=ALU.mult,
                )
            # ---- Cp broadcast to sbuf (used at every scan step)
            cp_flat = mam.tile([1, COLS * DS], F32, tag="flat", name="cp_flat")
            nc.sync.dma_start(
                out=cp_flat.rearrange("o (pb n) -> o pb n", n=DS),
                in_=dbc_sb[:, DTR + DS : DTR + 2 * DS],
            )
            cpb_ps = psB.tile([128, COLS * DS], F32, tag="bc", name="cpb_ps")
            for k in range(3):
                nc.tensor.matmul(
                    cpb_ps[:, 512 * k : 512 * (k + 1)], ones_row,
                    cp_flat[:, 512 * k : 512 * (k + 1)], start=True, stop=True,
                )
            cpb = mam.tile([128, COLS * DS], BF16)
            nc.vector.tensor_copy(out=cpb, in_=cpb_ps)

            # ---- selective scan over l; h[128, i, (b,n)]; y2[128, i, col]
            h = mam.tile([128, DIT, BL * DS], F32)
            ytmp = mam.tile([128, DIT, BL * DS], F32)
            y2 = mam.tile([128, DIT, COLS], F32)
            for l in range(L):
                blk = slice(l * BL * DS, (l + 1) * BL * DS)
                for i in range(DIT):
                    if l == 0:
                        nc.vector.tensor_copy(out=h[:, i, :], in_=dBu[:, i, blk])
                    else:
                        nc.vector.tensor_tensor(
                            out=h[:, i, :], in0=h[:, i, :], in1=dA[:, i, blk],
                            op=ALU.mult,
                        )
                        nc.vector.tensor_tensor(
                            out=h[:, i, :], in0=h[:, i, :], in1=dBu[:, i, blk],
                            op=ALU.add,
                        )
                    nc.vector.tensor_tensor(
                        out=ytmp[:, i, :], in0=h[:, i, :], in1=cpb[:, blk],
                        op=ALU.mult,
                    )
                    nc.vector.reduce_sum(
                        out=y2[:, i, l * BL : (l + 1) * BL],
                        in_=ytmp[:, i, :].rearrange("p (bb n) -> p bb n", n=DS),
                        axis=AX.X,
                    )
            # y2 += x * D_ssm
            for i in range(DIT):
                nc.vector.scalar_tensor_tensor(
                    out=y2[:, i, :], in0=xact[:, i, :], scalar=dssm2[:, i : i + 1],
                    in1=y2[:, i, :], op0=ALU.mult, op1=ALU.add,
                )

            # ---- vi2 = y2 @ W_out.T + Vi/48  (true pooled values)
            vi2 = vip.tile([128, NCT, COLS], F32, tag="v96", bufs=2)
            for mc in range(NCT):
                vp = psC.tile([128, COLS], F32, tag="ps96", name=f"vp{mc}")
                for i in range(DIT):
                    nc.tensor.matmul(
                        vp, woutT[:, i, mc * 128 : (mc + 1) * 128], y2[:, i, :],
                        start=(i == 0), stop=(i == DIT - 1),
                    )
                nc.vector.scalar_tensor_tensor(
                    out=vi2[:, mc, :], in0=Vi[:, mc, :], scalar=1.0 / POOL_W,
                    in1=vp, op0=ALU.mult, op1=ALU.add,
                )

            # ---- LN2
            nrm = vip.tile([128, NCT, COLS], F32, tag="v96", bufs=2)
            layer_norm(vi2, ln2g, ln2b, nrm)

            # ---- channel attention per stream
            att_t = [att_vis, att_inf]
            for s in range(2):
                mv = mam.tile([128, NCT, BL], F32, tag="mv", name=f"mv{s}")
                mx = mam.tile([128, NCT, BL], F32, tag="mx", name=f"mx{s}")
                for ci in range(NCT):
                    view = nrm[:, ci, :].rearrange(
                        "p (pp two bb) -> p two bb pp", two=2, bb=BL
                    )[:, s, :, :]
                    nc.vector.reduce_sum(out=mv[:, ci, :], in_=view, axis=AX.X)
                    nc.vector.reduce_max(out=mx[:, ci, :], in_=view, axis=AX.X)
                # fc1: relu(bn(v @ w1T)); mean path has 1/6 folded into absm
                h1m = mam.tile([128, BL], F32, tag="h1m", name=f"h1m{s}")
                h1x = mam.tile([128, BL], F32, tag="h1x", name=f"h1x{s}")
                for src, dst, sc in ((mv, h1m, absm), (mx, h1x, absx)):
                    hp = psC.tile([128, COLS], F32, tag="ps96", name=f"hp{s}")
                    for ci in range(NCT):
                        nc.tensor.matmul(
                            hp[:, 0:BL], aw1T[:, ci, :], src[:, ci, :],
                            start=(ci == 0), stop=(ci == NCT - 1),
                        )
                    nc.scalar.activation(
                        out=dst, in_=hp[:, 0:BL], func=AF.Relu, scale=sc, bias=abnb
                    )
                # fc2 (sum of mean/max paths) then sigmoid via exp
                for mc in range(NCT):
                    ap2 = psC.tile([128, COLS], F32, tag="ps96", name=f"ap{s}_{mc}")
                    nc.tensor.matmul(
                        ap2[:, 0:BL], aw2T[:, mc * 128 : (mc + 1) * 128], h1m,
                        start=True, stop=False,
                    )
                    nc.tensor.matmul(
                        ap2[:, 0:BL], aw2T[:, mc * 128 : (mc + 1) * 128], h1x,
                        start=False, stop=True,
                    )
                    nc.scalar.activation(
                        out=att_t[s][:, mc, :], in_=ap2[:, 0:BL], func=AF.Exp,
                        scale=-1.0,
                    )
                    nc.vector.tensor_scalar_add(
                        out=att_t[s][:, mc, :], in0=att_t[s][:, mc, :], scalar1=1.0
                    )
                    nc.vector.reciprocal(
                        out=att_t[s][:, mc, :], in_=att_t[s][:, mc, :]
                    )

            # ============================================================
            # Phase F: out = relu((W @ (a*fm))*s + b) per stream, per batch
            # ============================================================
            scl_s = [fvs, fis]
            scl_b = [fvb, fib]
            w_dram = [d_wvT, d_wiT]
            for s in range(2):
                wt = wpool.tile([128, NCT, C], BF16, tag="w", name=f"w{s}")
                for kc in range(NCT):
                    nc.sync.dma_start(out=wt[:, kc, :], in_=w_dram[s][:, kc, :])
                for b in range(BL):
                    ax = axpool.tile(
                        [128, NCT, HW], BF16, tag="ax", name=f"ax{s}_{b}"
                    )
                    for ci in range(NCT):
                        ft = stream.tile(
                            [128, HW], F32, tag="fm", name=f"ffm{s}_{b}_{ci}"
                        )
                        nc.sync.dma_start(
                            out=ft, in_=fm_d[s][b, ci * 128 : (ci + 1) * 128, :]
                        )
                        nc.vector.tensor_scalar_mul(
                            out=ax[:, ci, :], in0=ft,
                            scalar1=att_t[s][:, ci, b : b + 1],
                        )
                    for mc in range(NCT):
                        pp = psA.tile([128, HW], F32, tag="pp", name=f"pp{s}_{b}_{mc}")
                        for kc in range(NCT):
                            nc.tensor.matmul(
                                pp, wt[:, kc, mc * 128 : (mc + 1) * 128],
                                ax[:, kc, :],
                                start=(kc == 0), stop=(kc == NCT - 1),
                            )
                        ot = outp.tile([128, HW], F32, tag="ot", name=f"ot{s}_{b}_{mc}")
                        nc.scalar.activation(
                            out=ot, in_=pp, func=AF.Relu,
                            scale=scl_s[s][:, mc : mc + 1],
                            bias=scl_b[s][:, mc : mc + 1],
                        )
                        nc.sync.dma_start(
                            out=out_d[s][b, mc * 128 : (mc + 1) * 128, :], in_=ot
                        )

    nc.compile()
    return nc


def _host_prep(inputs):
    """Host-side weight layout prep. Returns dict of per-core-replicated arrays."""
    f32 = np.float32
    g = lambda k: np.asarray(inputs[k], dtype=f32)
    s_bn = f32(1.0 / np.sqrt(1.0 + EPS))

    def ctile(v):  # [C] -> [128, 16]
        return np.ascontiguousarray(v.reshape(NCT, 128).T)

    def dtile(v):  # [DI] -> [128, 2]
        return np.ascontiguousarray(v.reshape(DIT, 128).T)

    A = -np.exp(g("A_log"))  # [256, 16]
    prep = {
        "w_inT": np.ascontiguousarray(
            g("W_in").T.reshape(NCT, 128, DI).transpose(1, 0, 2)
        ),
        "wxT": np.ascontiguousarray(g("Wx").T.reshape(DIT, 128, 48).transpose(1, 0, 2)),
        "wdtT": np.ascontiguousarray(g("Wdt").T),
        "w_outT": np.ascontiguousarray(
            g("W_out").T.reshape(DIT, 128, C).transpose(1, 0, 2)
        ),
        "aw1T": np.ascontiguousarray(
            g("att_w1").T.reshape(NCT, 128, 128).transpose(1, 0, 2)
        ),
        "aw2T": np.ascontiguousarray(g("att_w2").T),
        "A3": np.ascontiguousarray(A.reshape(DIT, 128, DS).transpose(1, 0, 2)),
        "cw3": np.ascontiguousarray(
            g("conv_w")[:, 0, :].reshape(DIT, 128, 3).transpose(1, 0, 2)
        ),
        "ncb": dtile(-g("conv_b")),
        "bdt2": dtile(g("bdt")),
        "dssm2": dtile(g("D_ssm")),
        "ln1g": ctile(g("ln1_g")),
        "ln1b": ctile(g("ln1_b")),
        "ln2g": ctile(g("ln2_g")),
        "ln2b": ctile(g("ln2_b")),
        "absm": np.ascontiguousarray((g("att_bn_g") * s_bn / 6.0)[:, None]),
        "absx": np.ascontiguousarray((g("att_bn_g") * s_bn)[:, None]),
        "abnb": np.ascontiguousarray(g("att_bn_b")[:, None]),
        "wvT": np.ascontiguousarray(
            g("ffn_vis_w").T.reshape(NCT, 128, C).transpose(1, 0, 2)
        ).astype(ml_dtypes.bfloat16),
        "wiT": np.ascontiguousarray(
            g("ffn_inf_w").T.reshape(NCT, 128, C).transpose(1, 0, 2)
        ).astype(ml_dtypes.bfloat16),
        "fvs": ctile(g("ffn_vis_bn_g") * s_bn),
        "fvb": ctile(g("ffn_vis_b") * (g("ffn_vis_bn_g") * s_bn) + g("ffn_vis_bn_b")),
        "fis": ctile(g("ffn_inf_bn_g") * s_bn),
        "fib": ctile(g("ffn_inf_b") * (g("ffn_inf_bn_g") * s_bn) + g("ffn_inf_bn_b")),
    }
    return prep


def _get_runner():
    """Build the bass program once and wrap it in a reusable jitted callable."""
    if "runner" in _CACHE:
        return _CACHE["runner"]

    import jax
    import numpy as _np
    from jax.sharding import Mesh, PartitionSpec
    from jax.experimental.shard_map import shard_map
    import concourse.bacc as bacc
    import concourse.tile as tile
    from concourse import mybir, masks
    from concourse import bass2jax

    nc = _build(bacc, tile, mybir, masks)
    bass2jax.install_neuronx_cc_hook()

    pname = nc.partition_id_tensor.name if nc.partition_id_tensor else None
    in_names, out_names, out_avals, zero_shapes = [], [], [], []
    for alloc in nc.m.functions[0].allocations:
        if not isinstance(alloc, mybir.MemoryLocationSet):
            continue
        name = alloc.memorylocations[0].name
        if alloc.kind == "ExternalInput":
            if name != pname:
                in_names.append(name)
        elif alloc.kind == "ExternalOutput":
            out_names.append(name)
            shape = tuple(alloc.tensor_shape)
            dtype = mybir.dt.np(alloc.dtype)
            out_avals.append(jax.core.ShapedArray(shape, dtype))
            zero_shapes.append((shape, dtype))
    n_params = len(in_names)
    all_names = list(in_names) + list(out_names)
    if pname is not None:
        all_names.append(pname)

    def _body(*args):
        operands = list(args)
        if pname is not None:
            operands.append(bass2jax.partition_id_tensor())
        outs = bass2jax._bass_exec_p.bind(
            *operands,
            out_avals=tuple(out_avals),
            in_names=tuple(all_names),
            out_names=tuple(out_names),
            lowering_input_output_aliases=(),
            sim_require_finite=False,
            sim_require_nnan=False,
            nc=nc,
        )
        return tuple(outs)

    devices = jax.devices()[:N_CORES]
    mesh = Mesh(_np.asarray(devices), ("core",))
    specs = (PartitionSpec("core"),) * (n_params + len(out_names))
    fn = jax.jit(
        shard_map(
            _body,
            mesh=mesh,
            in_specs=specs,
            out_specs=(PartitionSpec("core"),) * len(out_names),
            check_rep=False,
        ),
        keep_unused=True,
    )
    runner = {
        "fn": fn,
        "in_names": in_names,
        "out_names": out_names,
        "zero_shapes": zero_shapes,
        "nc": nc,
    }
    _CACHE["runner"] = runner
    return runner


def kernel(**inputs):
    runner = _get_runner()
    prep = _host_prep(inputs)
    vis = np.asarray(inputs["vis_feat_map"], dtype=np.float32).reshape(B_FULL, C, HW)
    inf = np.asarray(inputs["inf_feat_map"], dtype=np.float32).reshape(B_FULL, C, HW)

    # global inputs: concat of per-core shards along axis 0
    per_in = {"vis": vis, "inf": inf}  # already [64, ...] = 8 cores x [8, ...]
    gin = []
    for name in runner["in_names"]:
        if name in per_in:
            gin.append(per_in[name])
        else:
            arr = prep[name]
            gin.append(np.broadcast_to(arr, (N_CORES,) + arr.shape).reshape(
                (N_CORES * arr.shape[0],) + arr.shape[1:]
            ))
    zeros = [
        np.zeros((N_CORES * s[0],) + tuple(s[1:]), dt)
        for (s, dt) in runner["zero_shapes"]
    ]
    outs = runner["fn"](*gin, *zeros)
    res = {
        name: np.asarray(outs[i]) for i, name in enumerate(runner["out_names"])
    }
    out_vis = res["out_vis"].reshape(B_FULL, C, H, W)
    out_inf = res["out_inf"].reshape(B_FULL, C, H, W)
    return (out_vis, out_inf)
